# revision 25
# baseline (speedup 1.0000x reference)
"""Multi-head attention TRN2 kernel (8 NeuronCores).

Sharding: data parallel on batch (B=2, 4 cores each), tensor parallel on
heads (4 of 16 heads per core; wq/wk/wv column-parallel, wo row-parallel).
Each core computes a partial [D, S] transposed output for its batch; the
host sums the 4 partials per batch, transposes, and adds bo.

All activations/weights fp16 (PE runs fp16 at 1 cycle/row, PSUM
accumulates fp32). Causal masking is multiplicative post-exp, and
diagonal key blocks are TRIMMED: query columns that are fully masked for
a key block are skipped in the logits matmul, the exp, and the attn@V
matmul (only the [128,128] triangle needs the mask multiply).

Structured for PE continuity (the PE clock ramps 1.2->2.4 GHz only after
~3us of uninterrupted work):
  - x^T inputs live in [128,2048] full-row tiles (4KB DMA lines), each
    DMA'd as four 32-partition strips on separate queues;
  - Q/K/V projections and the output projection are split into small
    "filler" thunks drained between attention pairs, so the PE always
    has independent work queued while the Scalar engine runs exp;
  - PSUM: 2x [128,1024] logits + 2x [65,512] attn-out + 2x [128,512]
    filler tiles = exactly 8 banks.

Attention per head pair / 512-query tile / 128-key-block pair: logits^T
-> exp (ACT) -> attn^T fp16 -> attn@V accumulates out^T [65,512] in
PSUM (row 64 = softmax denominator via a ones column in V). Head-half 0
is extracted on the Scalar engine, half 1 on DVE ([65,512] fp32->fp16
casts); gpsimd DMAs place the O rows in ot_sb and gather the fp16
denominator rows for reciprocal -> partition_broadcast -> one [128,512]
normalize multiply per (head pair, query tile).
"""

import numpy as np

import concourse.bass as bass
import concourse.mybir as mybir
import concourse.tile as tile
from concourse import bacc
from concourse.bass_utils import run_bass_kernel_spmd

B = 2
S = 2048
D_MODEL = 1024
NUM_HEADS = 16
DEPTH = 64
NEG = -1e9
N_CORES = 8
CORES_PER_BATCH = 4
HEADS_PER_CORE = 4           # 4 heads x depth 64 = 256 d_out columns per core
DC = HEADS_PER_CORE * DEPTH  # 256
QT = 512                     # query tile (4 tiles)
KB = 128                     # key block (16 blocks, processed in pairs)
NQT = S // QT
NKB = S // KB
NPAIR = NKB // 2
KIN = D_MODEL // 128         # 8 contraction chunks of 128

F32 = mybir.dt.float32
F16 = mybir.dt.float16

_cache = {}


class _Filler:
    def __init__(self):
        self.q = []

    def add(self, thunks):
        self.q.extend(thunks)

    def drain(self, n):
        for _ in range(min(n, len(self.q))):
            self.q.pop(0)()

    def drain_all(self):
        while self.q:
            self.q.pop(0)()


def _build(pair_plan, n_masks):
    """pair_plan[(t, pj)] = (desc0, desc1, mask_idx|None) with desc =
    None (skip) | (vlo, mask_cols|None); mask_cols = (cl, ch) col range
    of the half that needs the multiplicative mask."""
    nc = bacc.Bacc("TRN2", target_bir_lowering=False, debug=False,
                   num_devices=N_CORES)

    xqT = nc.dram_tensor("xqT", [D_MODEL, S], F16, kind="ExternalInput").ap()
    xkT = nc.dram_tensor("xkT", [D_MODEL, S], F16, kind="ExternalInput").ap()
    xvT = nc.dram_tensor("xvT", [D_MODEL, S], F16, kind="ExternalInput").ap()
    wq = nc.dram_tensor("wq", [D_MODEL, DC], F16, kind="ExternalInput").ap()
    wk = nc.dram_tensor("wk", [D_MODEL, DC], F16, kind="ExternalInput").ap()
    wv = nc.dram_tensor("wv", [D_MODEL, DC], F16, kind="ExternalInput").ap()
    wo = nc.dram_tensor("wo", [DC, D_MODEL], F16, kind="ExternalInput").ap()
    bq = nc.dram_tensor("bq", [128, 2], F32, kind="ExternalInput").ap()
    bk = nc.dram_tensor("bk", [128, 2], F32, kind="ExternalInput").ap()
    bv = nc.dram_tensor("bv", [128, DC], F32, kind="ExternalInput").ap()
    masks = nc.dram_tensor("masks", [max(n_masks, 1), KB, 2 * QT], F16,
                           kind="ExternalInput").ap()
    outT = nc.dram_tensor("outT", [D_MODEL, S], F16, kind="ExternalOutput").ap()

    with tile.TileContext(nc) as tc:
        import contextlib
        ctx = contextlib.ExitStack()
        with ctx:
            wpool = ctx.enter_context(tc.tile_pool(name="weights", bufs=1))
            xpool = ctx.enter_context(tc.tile_pool(name="xin", bufs=1))
            qkv = ctx.enter_context(tc.tile_pool(name="qkv", bufs=1))
            expp = ctx.enter_context(tc.tile_pool(name="expp", bufs=6))
            ostp = ctx.enter_context(tc.tile_pool(name="ostp", bufs=4))
            osp = ctx.enter_context(tc.tile_pool(name="osp", bufs=4))
            nrmp = ctx.enter_context(tc.tile_pool(name="nrmp", bufs=1))
            rowp = ctx.enter_context(tc.tile_pool(name="rowp", bufs=4))
            plg = ctx.enter_context(
                tc.tile_pool(name="plg", bufs=2, space="PSUM"))
            ppo = ctx.enter_context(
                tc.tile_pool(name="ppo", bufs=2, space="PSUM"))
            pfil = ctx.enter_context(
                tc.tile_pool(name="pfil", bufs=2, space="PSUM"))

            # ---- resident weights / constants ------------------------------
            wq_sb = wpool.tile([128, KIN, DC], F16, tag="wq")
            wk_sb = wpool.tile([128, KIN, DC], F16, tag="wk")
            wv_sb = wpool.tile([128, KIN, DC], F16, tag="wv")
            wo_sb = wpool.tile([128, 2, D_MODEL], F16, tag="wo")
            bq_sb = wpool.tile([128, 2], F32, tag="bq")
            bk_sb = wpool.tile([128, 2], F32, tag="bk")
            bv_sb = wpool.tile([128, DC], F32, tag="bv")

            def load_w(eng, w_sb, wdram, b_sb, bdram):
                for c in range(KIN):
                    eng.dma_start(w_sb[:, c, :],
                                  wdram[c * 128:(c + 1) * 128, :])
                if b_sb is not None:
                    eng.dma_start(b_sb[:], bdram[:])

            mask_sb = []
            for i in range(n_masks):
                mt = wpool.tile([KB, 2 * QT], F16, tag=f"mask{i}",
                                name=f"mask{i}")
                mask_sb.append(mt)

            # persistent activations
            qt_sb = [qkv.tile([128, S], F16, tag=f"qt{i}", name=f"qt{i}")
                     for i in range(2)]
            kt_sb = [qkv.tile([128, S], F16, tag=f"kt{i}", name=f"kt{i}")
                     for i in range(2)]
            v_sb = qkv.tile([128, NKB, HEADS_PER_CORE, DEPTH + 1], F16,
                            tag="v")
            ot_sb = [qkv.tile([128, S], F16, tag=f"ot{i}", name=f"ot{i}")
                     for i in range(2)]

            ones_f16 = wpool.tile([128, 1], F16, tag="ones")

            # denominator staging: rs collects raw fp16 denoms, rr = 1/rs
            rs_sb = nrmp.tile([128, HEADS_PER_CORE * NQT * 4], F16, tag="rs")
            rr_sb = nrmp.tile([128, HEADS_PER_CORE * NQT * 4], F32, tag="rr")

            # ---- input staging: quarter-column x tiles. DMA issue costs
            # ~0.7us of the issuing engine's sequencer per call, so calls
            # are spread across the sync/scalar/gpsimd/vector sequencers
            # and kept few; only the first chunks are strip-split for
            # startup latency.
            x_tiles = {}
            xdr = {"q": xqT, "k": xkT, "v": xvT}

            def emit_xdma(eng, p, qt, strip_ch=()):
                for ch in range(KIN):
                    xt = xpool.tile([128, QT], F16, tag=f"x{p}{qt}{ch}",
                                    name=f"x{p}{qt}{ch}")
                    nsp = 2 if ch in strip_ch else 1
                    for sp in range(nsp):
                        w = 128 // nsp
                        eng.dma_start(
                            xt[sp * w:(sp + 1) * w, :],
                            xdr[p][ch * 128 + sp * w:
                                   ch * 128 + (sp + 1) * w,
                                   qt * QT:(qt + 1) * QT])
                    x_tiles[(p, qt, ch)] = xt

            # ---- projections / output projection as filler thunks ---------
            def proj_quarter_thunks(p, qt):
                w_sb, b_sb, dst = {
                    "q": (wq_sb, bq_sb, qt_sb),
                    "k": (wk_sb, bk_sb, kt_sb)}[p]
                csl = slice(qt * QT, (qt + 1) * QT)
                thunks = []
                for m in (0, 1):
                    hold = {}

                    def t1(m=m, hold=hold):
                        fil = pfil.tile([128, 512], F32, tag="fil",
                                        name=f"pj{p}{qt}{m}")
                        hold["f"] = fil
                        for ch in range(4):
                            nc.tensor.matmul(
                                fil[:], w_sb[:, ch, m * 128:(m + 1) * 128],
                                x_tiles[(p, qt, ch)][:],
                                start=(ch == 0), stop=False)

                    def t2(m=m, hold=hold):
                        fil = hold["f"]
                        for ch in range(4, KIN):
                            nc.tensor.matmul(
                                fil[:], w_sb[:, ch, m * 128:(m + 1) * 128],
                                x_tiles[(p, qt, ch)][:],
                                start=False, stop=(ch == KIN - 1))
                        nc.vector.tensor_scalar_add(
                            dst[m][:, csl], fil[:], b_sb[:, m:m + 1])

                    thunks += [t1, t2]
                return thunks

            def v_quarter_thunks(qt):
                thunks = []
                for si in range(4):
                    sc = qt * 4 + si
                    hold = {}

                    def t1(qt=qt, si=si, sc=sc, hold=hold):
                        fil = pfil.tile([128, 512], F32, tag="fil",
                                        name=f"pv{sc}")
                        hold["f"] = fil
                        for ch in range(4):
                            nc.tensor.matmul(
                                fil[:, 0:DC],
                                x_tiles[("v", qt, ch)][:,
                                                       si * 128:
                                                       (si + 1) * 128],
                                wv_sb[:, ch, :],
                                start=(ch == 0), stop=False)

                    def t2(qt=qt, si=si, sc=sc, hold=hold):
                        fil = hold["f"]
                        for ch in range(4, KIN):
                            nc.tensor.matmul(
                                fil[:, 0:DC],
                                x_tiles[("v", qt, ch)][:,
                                                       si * 128:
                                                       (si + 1) * 128],
                                wv_sb[:, ch, :],
                                start=False, stop=(ch == KIN - 1))
                        nc.vector.tensor_add(
                            v_sb[:, sc, :, 0:DEPTH],
                            fil[:, 0:DC].rearrange("p (h d) -> p h d",
                                                   h=HEADS_PER_CORE),
                            bv_sb[:].rearrange("p (h d) -> p h d",
                                               h=HEADS_PER_CORE))

                    thunks += [t1, t2]
                return thunks

            def outproj_thunks(t, tail=False):
                csl = slice(t * QT, (t + 1) * QT)
                thunks = []
                for dt in range(8):
                    def th(dt=dt):
                        fil = pfil.tile([128, 512], F32, tag="fil",
                                        name=f"pp{dt}{t}")
                        for bi in range(2):
                            nc.tensor.matmul(
                                fil[:],
                                wo_sb[:, bi, dt * 128:(dt + 1) * 128],
                                ot_sb[bi][:, csl],
                                start=(bi == 0), stop=(bi == 1))
                        ost = osp.tile([128, QT], F16, tag="os",
                                       name=f"os{dt}{t}")
                        if tail and dt % 2 == 1:
                            nc.scalar.copy(ost[:], fil[:])
                        else:
                            nc.vector.tensor_copy(ost[:], fil[:])
                        nsp = 4 if tail else 2
                        for sp in range(nsp):
                            w = 128 // nsp
                            nc.sync.dma_start(
                                outT[dt * 128 + sp * w:
                                     dt * 128 + (sp + 1) * w, csl],
                                ost[sp * w:(sp + 1) * w, :])

                    thunks.append(th)
                return thunks

            # ---- attention -------------------------------------------------
            def emit_attention(bi, t, F=None):
                q0 = t * QT
                pairs = []
                for pj in range(NPAIR):
                    d0, d1, mi = pair_plan[(t, pj)]
                    if d0 is not None or d1 is not None:
                        pairs.append((pj, d0, d1, mi))
                # first valid half must cover the full query tile so its
                # attn@V accumulation initializes all of po
                n_valid = sum((d0 is not None) + (d1 is not None)
                              for _, d0, d1, _ in pairs)
                po = {}
                n_av = {0: 0, 1: 0}
                for hp in range(2):
                    po[hp] = ppo.tile([DEPTH + 1, QT], F32, tag="po",
                                      name=f"po{bi}{t}{hp}")
                exps = {}

                def emit_av(i):
                    pj, d0, d1, _ = pairs[i]
                    et = exps[i]
                    for hp in range(2):
                        h = 2 * bi + hp
                        for half, d in ((0, d0), (1, d1)):
                            if d is None:
                                continue
                            vlo = d[0]
                            kb = 2 * pj + half
                            nc.tensor.matmul(
                                po[hp][:, vlo:QT],
                                v_sb[:, kb, h, :],
                                et[hp][:, half * QT + vlo:(half + 1) * QT],
                                start=(n_av[hp] == 0),
                                stop=(n_av[hp] == n_valid - 1))
                            n_av[hp] += 1

                for i, (pj, d0, d1, mi) in enumerate(pairs):
                    lg = {}
                    for hp in range(2):
                        lg[hp] = plg.tile(
                            [128, 1024], F32, tag="lg",
                            name=f"lg{bi}{t}{pj}{hp}")
                    for half, d in ((0, d0), (1, d1)):
                        if d is None:
                            continue
                        vlo = d[0]
                        kb = 2 * pj + half
                        for hp in range(2):
                            prow = slice(hp * 64, hp * 64 + 64)
                            nc.tensor.matmul(
                                lg[hp][:, half * QT + vlo:(half + 1) * QT],
                                kt_sb[bi][prow, kb * KB:(kb + 1) * KB],
                                qt_sb[bi][prow, q0 + vlo:q0 + QT],
                                start=True, stop=True)
                    et = {}
                    for hp in range(2):
                        et[hp] = expp.tile([128, 1024], F16, tag="exp",
                                           name=f"et{bi}{t}{pj}{hp}")
                        if (d0 is not None and d1 is not None
                                and d0[0] == 0 and d1[0] == 0):
                            nc.scalar.activation(
                                et[hp][:], lg[hp][:],
                                mybir.ActivationFunctionType.Exp)
                        else:
                            for half, d in ((0, d0), (1, d1)):
                                if d is None:
                                    continue
                                hs = slice(half * QT + d[0],
                                           (half + 1) * QT)
                                nc.scalar.activation(
                                    et[hp][:, hs], lg[hp][:, hs],
                                    mybir.ActivationFunctionType.Exp)
                        if mi is not None:
                            for half, d in ((0, d0), (1, d1)):
                                if d is None or d[1] is None:
                                    continue
                                cl, ch = d[1]
                                ms = slice(half * QT + cl, half * QT + ch)
                                nc.vector.tensor_mul(
                                    et[hp][:, ms], et[hp][:, ms],
                                    mask_sb[mi][:, ms])
                    exps[i] = et
                    if F is not None:
                        F.drain(2 if len(F.q) > 10 else 1)
                    if i > 0:
                        emit_av(i - 1)
                if pairs:
                    emit_av(len(pairs) - 1)
                if F is not None:
                    F.drain(2)

                # extract O (unnormalized) + fp16 denominator row in one
                # [65,512] cast per head-half (hp0 on ACT, hp1 on DVE);
                # gpsimd DMAs place/gather.
                for hp in range(2):
                    h = 2 * bi + hp
                    ht = h * NQT + t
                    oh = ostp.tile([DEPTH + 1, QT], F16, tag="ost",
                                   name=f"oh{bi}{t}{hp}")
                    if hp == 0:
                        nc.scalar.copy(oh[:], po[hp][:])
                    else:
                        nc.vector.tensor_copy(oh[:], po[hp][:])
                    nc.gpsimd.dma_start(
                        ot_sb[bi][hp * 64:hp * 64 + 64, q0:q0 + QT],
                        oh[0:DEPTH, :])
                    src = oh[DEPTH:DEPTH + 1, :].rearrange(
                        "o (p j) -> o p j", j=4)
                    nc.gpsimd.dma_start(rs_sb[:, ht * 4:(ht + 1) * 4], src)

            # normalize in two phases: prep (gathers + reciprocal +
            # partition broadcast; cheap queue entries, long latency) right
            # after the attention block, and the [128,512] multiply much
            # later -- so the multiply never head-of-line blocks the next
            # attention block's DVE work while the broadcast chain runs.
            bcbs = {}

            def emit_norm_prep(bi, t):
                bcb = rowp.tile([128, QT], F32, tag="bcb",
                                name=f"bcb{bi}{t}")
                bcbs[(bi, t)] = bcb
                for hp in range(2):
                    h = 2 * bi + hp
                    c0 = (h * NQT + t) * 4
                    nc.vector.reciprocal(rr_sb[:, c0:c0 + 4],
                                         rs_sb[:, c0:c0 + 4])
                    rowh = rowp.tile([1, QT], F32, tag="rowh",
                                     name=f"rowh{bi}{t}{hp}")
                    nc.gpsimd.dma_start(
                        rowh[:].rearrange("o (p j) -> o p j", j=4),
                        rr_sb[:, c0:c0 + 4])
                    if hp == 0:
                        nc.gpsimd.partition_broadcast(bcb[0:64, :], rowh[:])
                    else:
                        tmp = rowp.tile([64, QT], F32, tag="tmp",
                                        name=f"tmp{bi}{t}")
                        nc.gpsimd.partition_broadcast(tmp[:], rowh[:])
                        nc.gpsimd.dma_start(bcb[64:128, :], tmp[:])

            def emit_norm_mul(bi, t, split=False):
                bcb = bcbs.pop((bi, t))
                csl = slice(t * QT, (t + 1) * QT)
                if split:
                    for hp in range(2):
                        rs = slice(hp * 64, (hp + 1) * 64)
                        nc.vector.tensor_mul(ot_sb[bi][rs, csl],
                                             ot_sb[bi][rs, csl],
                                             bcb[rs, :])
                else:
                    nc.vector.tensor_mul(ot_sb[bi][:, csl],
                                         ot_sb[bi][:, csl], bcb[:])

            # ---- driver ----------------------------------------------------
            # DMA issue spread across engine sequencers, need-ordered:
            #   sync:   wq, xq0/1 (+masks, wo, later xq3/xk3/xv3, stores)
            #   scalar: wk, xk0/1 (all before the first exp)
            #   gpsimd: wv, xv0/1 (all before its first extraction DMA)
            #   vector: xq2/xk2/xv2 (after the warm-up bias-adds)
            load_w(nc.sync, wq_sb, wq, bq_sb, bq)
            emit_xdma(nc.sync, "q", 0, strip_ch=(0, 1, 2, 3))
            load_w(nc.scalar, wk_sb, wk, bk_sb, bk)
            emit_xdma(nc.scalar, "k", 0, strip_ch=(0, 1, 2, 3))
            load_w(nc.gpsimd, wv_sb, wv, bv_sb, bv)
            emit_xdma(nc.gpsimd, "v", 0)
            emit_xdma(nc.sync, "q", 1)
            emit_xdma(nc.scalar, "k", 1)
            emit_xdma(nc.gpsimd, "v", 1)
            for i in range(n_masks):
                nc.sync.dma_start(mask_sb[i][:], masks[i])
            for c in range(2):
                nc.sync.dma_start(wo_sb[:, c, :], wo[c * 128:(c + 1) * 128, :])

            nc.vector.memset(ones_f16[:], 1.0)
            nc.vector.tensor_copy(
                v_sb[:, :, :, DEPTH:DEPTH + 1],
                ones_f16[:, None, None, :].broadcast_to(
                    [128, NKB, HEADS_PER_CORE, 1]))

            F = _Filler()
            # dedicated warm-up: keeps PE streaming while xv lands
            for th in (proj_quarter_thunks("q", 0)
                       + proj_quarter_thunks("k", 0)
                       + proj_quarter_thunks("q", 1)
                       + proj_quarter_thunks("k", 1)
                       + v_quarter_thunks(0)):
                th()
            emit_xdma(nc.sync, "q", 2)
            emit_xdma(nc.sync, "k", 2)
            emit_xdma(nc.sync, "v", 2)

            F.add(v_quarter_thunks(1))
            F.add(proj_quarter_thunks("q", 2))
            emit_attention(0, 0, F)
            emit_norm_prep(0, 0)
            emit_xdma(nc.sync, "q", 3)
            emit_xdma(nc.sync, "k", 3)
            emit_attention(1, 0, F)
            emit_norm_prep(1, 0)
            emit_xdma(nc.sync, "v", 3)
            F.drain_all()        # quarter 1 (K/V) done before att t=1

            F.add(proj_quarter_thunks("k", 2))
            F.add(v_quarter_thunks(2))
            F.add(proj_quarter_thunks("q", 3))
            emit_attention(0, 1, F)
            emit_norm_prep(0, 1)
            emit_attention(1, 1, F)
            emit_norm_prep(1, 1)
            F.drain_all()        # quarter 2 complete before att t=2
            emit_norm_mul(0, 0)
            emit_norm_mul(1, 0)

            F.add(proj_quarter_thunks("k", 3))
            F.add(v_quarter_thunks(3))
            F.add(outproj_thunks(0))
            emit_attention(0, 2, F)
            emit_norm_prep(0, 2)
            emit_attention(1, 2, F)
            emit_norm_prep(1, 2)
            F.drain_all()        # quarter 3 + outproj t=0 done
            emit_norm_mul(0, 1)
            emit_norm_mul(1, 1)

            F.add(outproj_thunks(1))
            emit_attention(0, 3, F)
            emit_norm_prep(0, 3)
            emit_norm_mul(0, 2)
            emit_norm_mul(1, 2)
            F.add(outproj_thunks(2))
            emit_attention(1, 3, F)
            emit_norm_prep(1, 3)
            F.drain_all()
            emit_norm_mul(0, 3)
            emit_norm_mul(1, 3, split=True)
            for th in outproj_thunks(3, tail=True):
                th()

    nc.compile()
    return nc


def _plan_from_mask(mask):
    """Classify (qtile, kblock-pair) blocks; returns plan + unique tiles.

    plan[(t, pj)] = (desc0, desc1, mask_idx|None), desc = None (fully
    masked) | (vlo, mask_cols|None): vlo = 128-aligned count of leading
    fully-masked query columns (trimmed everywhere), mask_cols = (cl,ch)
    128-aligned query col range (>= vlo) needing the multiplicative
    mask. Mask tiles are [128, 1024] exp(NEG*mask) of transposed halves.
    """
    m = np.asarray(mask, dtype=np.float64).reshape(S, S)  # [q, k]
    plan = {}
    tiles = []
    keys = {}
    for t in range(NQT):
        first_valid = True
        for pj in range(NPAIR):
            descs = []
            tile_halves = [None, None]
            for half in range(2):
                kb = 2 * pj + half
                blk = m[t * QT:(t + 1) * QT, kb * KB:(kb + 1) * KB]  # [q,k]
                if (blk >= 1.0).all():
                    descs.append(None)
                    continue
                row_full = (blk >= 1.0).all(axis=1)  # fully-masked q rows
                vlo = 0
                while vlo + 128 <= QT and row_full[vlo:vlo + 128].all():
                    vlo += 128
                if first_valid:
                    vlo = 0
                first_valid = False
                rest = blk[vlo:, :]
                if not rest.any():
                    descs.append((vlo, None))
                    continue
                idx = np.nonzero(blk[vlo:, :].any(axis=1))[0] + vlo
                cl = (idx.min() // 128) * 128
                ch = ((idx.max() // 128) + 1) * 128
                descs.append((vlo, (int(cl), int(ch))))
                tile_halves[half] = np.exp(
                    np.float64(NEG) * blk.T).astype(np.float32)
            d0, d1 = descs
            if d0 is None and d1 is None:
                plan[(t, pj)] = (None, None, None)
                continue
            if tile_halves[0] is None and tile_halves[1] is None:
                plan[(t, pj)] = (d0, d1, None)
                continue
            pair = np.zeros((KB, 2 * QT), np.float32)
            for half in range(2):
                hv = tile_halves[half]
                if hv is not None:
                    pair[:, half * QT:(half + 1) * QT] = hv
                elif descs[half] is not None:
                    pair[:, half * QT:(half + 1) * QT] = 1.0
            key = pair.tobytes()
            if key not in keys:
                keys[key] = len(tiles)
                tiles.append(pair)
            plan[(t, pj)] = (d0, d1, keys[key])
    return plan, tiles


def kernel(query, key_in, value, mask, wq, bq, wk, bk, wv, bv, wo, bo):
    query = np.asarray(query, dtype=np.float32)
    key_in = np.asarray(key_in, dtype=np.float32)
    value = np.asarray(value, dtype=np.float32)
    wq = np.asarray(wq, dtype=np.float32)
    wk = np.asarray(wk, dtype=np.float32)
    wv = np.asarray(wv, dtype=np.float32)
    wo = np.asarray(wo, dtype=np.float32)
    bq = np.asarray(bq, dtype=np.float32)
    bk = np.asarray(bk, dtype=np.float32)
    bv = np.asarray(bv, dtype=np.float32)
    bo = np.asarray(bo, dtype=np.float32)

    plan, mask_tiles = _plan_from_mask(mask)
    sig = tuple(sorted(plan.items()))
    if sig not in _cache:
        _cache[sig] = _build(plan, len(mask_tiles))
    nc = _cache[sig]

    scale = 1.0 / np.sqrt(np.float32(DEPTH))
    masks_arr = (np.stack(mask_tiles).astype(np.float16) if mask_tiles
                 else np.zeros((1, KB, 2 * QT), np.float16))

    xT = {}
    for b in range(B):
        xT[("q", b)] = np.ascontiguousarray(query[b].T).astype(np.float16)
        xT[("k", b)] = np.ascontiguousarray(key_in[b].T).astype(np.float16)
        xT[("v", b)] = np.ascontiguousarray(value[b].T).astype(np.float16)

    in_maps = []
    for c in range(N_CORES):
        b = c // CORES_PER_BATCH
        g = c % CORES_PER_BATCH
        sl = slice(g * DC, (g + 1) * DC)
        in_maps.append({
            "xqT": xT[("q", b)],
            "xkT": xT[("k", b)],
            "xvT": xT[("v", b)],
            "wq": (np.ascontiguousarray(wq[:, sl]) * scale).astype(np.float16),
            "wk": np.ascontiguousarray(wk[:, sl]).astype(np.float16),
            "wv": np.ascontiguousarray(wv[:, sl]).astype(np.float16),
            "wo": np.ascontiguousarray(wo[sl, :]).astype(np.float16),
            "bq": np.ascontiguousarray((bq[sl] * scale).reshape(2, 128).T),
            "bk": np.ascontiguousarray(bk[sl].reshape(2, 128).T),
            "bv": np.ascontiguousarray(
                np.broadcast_to(bv[sl], (128, DC))),
            "masks": masks_arr,
        })

    res = run_bass_kernel_spmd(nc, in_maps, list(range(N_CORES)))
    kernel.last_results = res

    out = np.zeros((B, S, D_MODEL), np.float32)
    for b in range(B):
        acc = np.zeros((D_MODEL, S), np.float32)
        for g in range(CORES_PER_BATCH):
            acc += res.results[b * CORES_PER_BATCH + g]["outT"].astype(
                np.float32)
        out[b] = acc.T + bo
    return out


# revision 37
# speedup vs baseline: 1.0114x; 1.0114x over previous
"""Multi-head attention TRN2 kernel (8 NeuronCores).

Sharding: data parallel on batch (B=2, 4 cores each), tensor parallel on
heads (4 of 16 heads per core; wq/wk/wv column-parallel, wo row-parallel).
Each core computes a partial [D, S] transposed output for its batch; the
host sums the 4 partials per batch, transposes, and adds bo.

All activations/weights fp16 (PE runs fp16 at 1 cycle/row, PSUM
accumulates fp32). Causal masking is multiplicative post-exp, and
diagonal key blocks are TRIMMED: query columns that are fully masked for
a key block are skipped in the logits matmul, the exp, and the attn@V
matmul (only the [128,128] triangle needs the mask multiply).

Structured for PE continuity (the PE clock ramps 1.2->2.4 GHz only after
~3us of uninterrupted work):
  - x^T inputs live in [128,2048] full-row tiles (4KB DMA lines), each
    DMA'd as four 32-partition strips on separate queues;
  - Q/K/V projections and the output projection are split into small
    "filler" thunks drained between attention pairs, so the PE always
    has independent work queued while the Scalar engine runs exp;
  - PSUM: 2x [128,1024] logits + 2x [65,512] attn-out + 2x [128,512]
    filler tiles = exactly 8 banks.

Attention per head pair / 512-query tile / 128-key-block pair: logits^T
-> exp (ACT) -> attn^T fp16 -> attn@V accumulates out^T [65,512] in
PSUM (row 64 = softmax denominator via a ones column in V). Head-half 0
is extracted on the Scalar engine, half 1 on DVE ([65,512] fp32->fp16
casts); gpsimd DMAs place the O rows in ot_sb and gather the fp16
denominator rows for reciprocal -> partition_broadcast -> one [128,512]
normalize multiply per (head pair, query tile).
"""

import numpy as np

import concourse.bass as bass
import concourse.mybir as mybir
import concourse.tile as tile
from concourse import bacc
from concourse.bass_utils import run_bass_kernel_spmd

B = 2
S = 2048
D_MODEL = 1024
NUM_HEADS = 16
DEPTH = 64
NEG = -1e9
N_CORES = 8
CORES_PER_BATCH = 4
HEADS_PER_CORE = 4           # 4 heads x depth 64 = 256 d_out columns per core
DC = HEADS_PER_CORE * DEPTH  # 256
QT = 512                     # query tile (4 tiles)
KB = 128                     # key block (16 blocks, processed in pairs)
NQT = S // QT
NKB = S // KB
NPAIR = NKB // 2
KIN = D_MODEL // 128         # 8 contraction chunks of 128

F32 = mybir.dt.float32
F16 = mybir.dt.float16

_cache = {}


class _Filler:
    def __init__(self):
        self.q = []

    def add(self, thunks):
        self.q.extend(thunks)

    def drain(self, n):
        for _ in range(min(n, len(self.q))):
            self.q.pop(0)()

    def drain_all(self):
        while self.q:
            self.q.pop(0)()


def _build(pair_plan, n_masks):
    """pair_plan[(t, pj)] = (desc0, desc1, mask_idx|None) with desc =
    None (skip) | (vlo, mask_cols|None); mask_cols = (cl, ch) col range
    of the half that needs the multiplicative mask."""
    nc = bacc.Bacc("TRN2", target_bir_lowering=False, debug=False,
                   num_devices=N_CORES)

    xqT = nc.dram_tensor("xqT", [D_MODEL, S], F16, kind="ExternalInput").ap()
    xkT = nc.dram_tensor("xkT", [D_MODEL, S], F16, kind="ExternalInput").ap()
    xvT = nc.dram_tensor("xvT", [D_MODEL, S], F16, kind="ExternalInput").ap()
    wq = nc.dram_tensor("wq", [D_MODEL, DC], F16, kind="ExternalInput").ap()
    wk = nc.dram_tensor("wk", [D_MODEL, DC], F16, kind="ExternalInput").ap()
    wv = nc.dram_tensor("wv", [D_MODEL, DC], F16, kind="ExternalInput").ap()
    wo = nc.dram_tensor("wo", [DC, D_MODEL], F16, kind="ExternalInput").ap()
    bq = nc.dram_tensor("bq", [128, 2], F32, kind="ExternalInput").ap()
    bk = nc.dram_tensor("bk", [128, 2], F32, kind="ExternalInput").ap()
    bv = nc.dram_tensor("bv", [128, DC], F32, kind="ExternalInput").ap()
    masks = nc.dram_tensor("masks", [max(n_masks, 1), KB, 2 * QT], F16,
                           kind="ExternalInput").ap()
    outT = nc.dram_tensor("outT", [D_MODEL, S], F16, kind="ExternalOutput").ap()

    with tile.TileContext(nc) as tc:
        import contextlib
        ctx = contextlib.ExitStack()
        with ctx:
            wpool = ctx.enter_context(tc.tile_pool(name="weights", bufs=1))
            xpool = ctx.enter_context(tc.tile_pool(name="xin", bufs=1))
            qkv = ctx.enter_context(tc.tile_pool(name="qkv", bufs=1))
            expp = ctx.enter_context(tc.tile_pool(name="expp", bufs=6))
            ostp = ctx.enter_context(tc.tile_pool(name="ostp", bufs=4))
            osp = ctx.enter_context(tc.tile_pool(name="osp", bufs=4))
            nrmp = ctx.enter_context(tc.tile_pool(name="nrmp", bufs=1))
            rowp = ctx.enter_context(tc.tile_pool(name="rowp", bufs=4))
            plg = ctx.enter_context(
                tc.tile_pool(name="plg", bufs=2, space="PSUM"))
            ppo = ctx.enter_context(
                tc.tile_pool(name="ppo", bufs=2, space="PSUM"))
            pfil = ctx.enter_context(
                tc.tile_pool(name="pfil", bufs=2, space="PSUM"))

            # ---- resident weights / constants ------------------------------
            wq_sb = wpool.tile([128, KIN, DC], F16, tag="wq")
            wk_sb = wpool.tile([128, KIN, DC], F16, tag="wk")
            wv_sb = wpool.tile([128, KIN, DC], F16, tag="wv")
            wo_sb = wpool.tile([128, 2, D_MODEL], F16, tag="wo")
            bq_sb = wpool.tile([128, 2], F32, tag="bq")
            bk_sb = wpool.tile([128, 2], F32, tag="bk")
            bv_sb = wpool.tile([128, DC], F32, tag="bv")

            def load_w(eng, w_sb, wdram, b_sb, bdram):
                for c in range(KIN):
                    eng.dma_start(w_sb[:, c, :],
                                  wdram[c * 128:(c + 1) * 128, :])
                if b_sb is not None:
                    eng.dma_start(b_sb[:], bdram[:])

            mask_sb = []
            for i in range(n_masks):
                mt = wpool.tile([KB, 2 * QT], F16, tag=f"mask{i}",
                                name=f"mask{i}")
                mask_sb.append(mt)

            # persistent activations
            qt_sb = [qkv.tile([128, S], F16, tag=f"qt{i}", name=f"qt{i}")
                     for i in range(2)]
            kt_sb = [qkv.tile([128, S], F16, tag=f"kt{i}", name=f"kt{i}")
                     for i in range(2)]
            v_sb = qkv.tile([128, NKB, HEADS_PER_CORE, DEPTH + 1], F16,
                            tag="v")
            ot_sb = [qkv.tile([128, S], F16, tag=f"ot{i}", name=f"ot{i}")
                     for i in range(2)]

            ones_f16 = wpool.tile([128, 1], F16, tag="ones")
            # selector rows (partition 64) for the tail broadcast matmuls:
            # cols 0:64 of sel[...,0:128] are 1 (head-half 0 target rows),
            # cols 192:256 of sel[...,128:256] are 1 (head-half 1 rows)
            sel64 = wpool.tile([DEPTH + 1, 2 * 128], F32, tag="sel64")

            # denominator staging: rs collects raw fp16 denoms, rr = 1/rs
            rs_sb = nrmp.tile([128, HEADS_PER_CORE * NQT * 4], F16, tag="rs")
            rr_sb = nrmp.tile([128, HEADS_PER_CORE * NQT * 4], F32, tag="rr")

            # ---- input staging: quarter-column x tiles. DMA issue costs
            # ~0.7us of the issuing engine's sequencer per call, so calls
            # are spread across the sync/scalar/gpsimd/vector sequencers
            # and kept few; only the first chunks are strip-split for
            # startup latency.
            x_tiles = {}
            xdr = {"q": xqT, "k": xkT, "v": xvT}

            def emit_xdma(eng, p, qt, strip_ch=()):
                for ch in range(KIN):
                    xt = xpool.tile([128, QT], F16, tag=f"x{p}{qt}{ch}",
                                    name=f"x{p}{qt}{ch}")
                    nsp = 2 if ch in strip_ch else 1
                    for sp in range(nsp):
                        w = 128 // nsp
                        eng.dma_start(
                            xt[sp * w:(sp + 1) * w, :],
                            xdr[p][ch * 128 + sp * w:
                                   ch * 128 + (sp + 1) * w,
                                   qt * QT:(qt + 1) * QT])
                    x_tiles[(p, qt, ch)] = xt

            # ---- projections / output projection as filler thunks ---------
            def proj_quarter_thunks(p, qt):
                w_sb, b_sb, dst = {
                    "q": (wq_sb, bq_sb, qt_sb),
                    "k": (wk_sb, bk_sb, kt_sb)}[p]
                csl = slice(qt * QT, (qt + 1) * QT)
                thunks = []
                for m in (0, 1):
                    hold = {}

                    def t1(m=m, hold=hold):
                        fil = pfil.tile([128, 512], F32, tag="fil",
                                        name=f"pj{p}{qt}{m}")
                        hold["f"] = fil
                        for ch in range(4):
                            nc.tensor.matmul(
                                fil[:], w_sb[:, ch, m * 128:(m + 1) * 128],
                                x_tiles[(p, qt, ch)][:],
                                start=(ch == 0), stop=False)

                    def t2(m=m, hold=hold):
                        fil = hold["f"]
                        for ch in range(4, KIN):
                            nc.tensor.matmul(
                                fil[:], w_sb[:, ch, m * 128:(m + 1) * 128],
                                x_tiles[(p, qt, ch)][:],
                                start=False, stop=(ch == KIN - 1))
                        nc.vector.tensor_scalar_add(
                            dst[m][:, csl], fil[:], b_sb[:, m:m + 1])

                    thunks += [t1, t2]
                return thunks

            def v_quarter_thunks(qt):
                thunks = []
                for si in range(4):
                    sc = qt * 4 + si
                    hold = {}

                    def t1(qt=qt, si=si, sc=sc, hold=hold):
                        fil = pfil.tile([128, 512], F32, tag="fil",
                                        name=f"pv{sc}")
                        hold["f"] = fil
                        for ch in range(4):
                            nc.tensor.matmul(
                                fil[:, 0:DC],
                                x_tiles[("v", qt, ch)][:,
                                                       si * 128:
                                                       (si + 1) * 128],
                                wv_sb[:, ch, :],
                                start=(ch == 0), stop=False)

                    def t2(qt=qt, si=si, sc=sc, hold=hold):
                        fil = hold["f"]
                        for ch in range(4, KIN):
                            nc.tensor.matmul(
                                fil[:, 0:DC],
                                x_tiles[("v", qt, ch)][:,
                                                       si * 128:
                                                       (si + 1) * 128],
                                wv_sb[:, ch, :],
                                start=False, stop=(ch == KIN - 1))
                        nc.vector.tensor_add(
                            v_sb[:, sc, :, 0:DEPTH],
                            fil[:, 0:DC].rearrange("p (h d) -> p h d",
                                                   h=HEADS_PER_CORE),
                            bv_sb[:].rearrange("p (h d) -> p h d",
                                               h=HEADS_PER_CORE))

                    thunks += [t1, t2]
                return thunks

            def outproj_thunks(t, tail=False):
                csl = slice(t * QT, (t + 1) * QT)
                thunks = []
                for dt in range(8):
                    def th(dt=dt):
                        fil = pfil.tile([128, 512], F32, tag="fil",
                                        name=f"pp{dt}{t}")
                        for bi in range(2):
                            nc.tensor.matmul(
                                fil[:],
                                wo_sb[:, bi, dt * 128:(dt + 1) * 128],
                                ot_sb[bi][:, csl],
                                start=(bi == 0), stop=(bi == 1))
                        ost = osp.tile([128, QT], F16, tag="os",
                                       name=f"os{dt}{t}")
                        if tail and dt % 2 == 1:
                            nc.scalar.copy(ost[:], fil[:])
                        else:
                            nc.vector.tensor_copy(ost[:], fil[:])
                        nsp = 4 if tail else 2
                        for sp in range(nsp):
                            w = 128 // nsp
                            nc.sync.dma_start(
                                outT[dt * 128 + sp * w:
                                     dt * 128 + (sp + 1) * w, csl],
                                ost[sp * w:(sp + 1) * w, :])

                    thunks.append(th)
                return thunks

            # ---- attention -------------------------------------------------
            last_oh = {}

            def emit_attention(bi, t, F=None, barrier_at=None):
                q0 = t * QT
                pairs = []
                for pj in range(NPAIR):
                    d0, d1, mi = pair_plan[(t, pj)]
                    if d0 is not None or d1 is not None:
                        pairs.append((pj, d0, d1, mi))
                # first valid half must cover the full query tile so its
                # attn@V accumulation initializes all of po
                n_valid = sum((d0 is not None) + (d1 is not None)
                              for _, d0, d1, _ in pairs)
                po = {}
                n_av = {0: 0, 1: 0}
                for hp in range(2):
                    po[hp] = ppo.tile([DEPTH + 1, QT], F32, tag="po",
                                      name=f"po{bi}{t}{hp}")
                exps = {}

                def emit_av(i):
                    pj, d0, d1, _ = pairs[i]
                    et = exps[i]
                    for hp in range(2):
                        h = 2 * bi + hp
                        for half, d in ((0, d0), (1, d1)):
                            if d is None:
                                continue
                            vlo = d[0]
                            kb = 2 * pj + half
                            nc.tensor.matmul(
                                po[hp][:, vlo:QT],
                                v_sb[:, kb, h, :],
                                et[hp][:, half * QT + vlo:(half + 1) * QT],
                                start=(n_av[hp] == 0),
                                stop=(n_av[hp] == n_valid - 1))
                            n_av[hp] += 1

                for i, (pj, d0, d1, mi) in enumerate(pairs):
                    if F is not None and barrier_at is not None \
                            and i == barrier_at:
                        F.drain_all()
                    lg = {}
                    for hp in range(2):
                        lg[hp] = plg.tile(
                            [128, 1024], F32, tag="lg",
                            name=f"lg{bi}{t}{pj}{hp}")
                    for half, d in ((0, d0), (1, d1)):
                        if d is None:
                            continue
                        vlo = d[0]
                        kb = 2 * pj + half
                        for hp in range(2):
                            prow = slice(hp * 64, hp * 64 + 64)
                            nc.tensor.matmul(
                                lg[hp][:, half * QT + vlo:(half + 1) * QT],
                                kt_sb[bi][prow, kb * KB:(kb + 1) * KB],
                                qt_sb[bi][prow, q0 + vlo:q0 + QT],
                                start=True, stop=True)
                    et = {}
                    for hp in range(2):
                        et[hp] = expp.tile([128, 1024], F16, tag="exp",
                                           name=f"et{bi}{t}{pj}{hp}")
                        if (d0 is not None and d1 is not None
                                and d0[0] == 0 and d1[0] == 0):
                            nc.scalar.activation(
                                et[hp][:], lg[hp][:],
                                mybir.ActivationFunctionType.Exp)
                        else:
                            for half, d in ((0, d0), (1, d1)):
                                if d is None:
                                    continue
                                hs = slice(half * QT + d[0],
                                           (half + 1) * QT)
                                nc.scalar.activation(
                                    et[hp][:, hs], lg[hp][:, hs],
                                    mybir.ActivationFunctionType.Exp)
                        if mi is not None:
                            for half, d in ((0, d0), (1, d1)):
                                if d is None or d[1] is None:
                                    continue
                                cl, ch = d[1]
                                ms = slice(half * QT + cl, half * QT + ch)
                                nc.vector.tensor_mul(
                                    et[hp][:, ms], et[hp][:, ms],
                                    mask_sb[mi][:, ms])
                    exps[i] = et
                    if F is not None:
                        F.drain(2 if len(F.q) > 10 else 1)
                    if i > 0:
                        emit_av(i - 1)
                if pairs:
                    emit_av(len(pairs) - 1)
                if F is not None:
                    F.drain(2)

                # extract O (unnormalized) + fp16 denominator row in one
                # [65,512] cast per head-half (hp0 on ACT, hp1 on DVE);
                # gpsimd DMAs place/gather.
                for hp in range(2):
                    h = 2 * bi + hp
                    ht = h * NQT + t
                    oh = ostp.tile([DEPTH + 1, QT], F16, tag="ost",
                                   name=f"oh{bi}{t}{hp}")
                    if hp == 0:
                        nc.scalar.copy(oh[:], po[hp][:])
                    else:
                        nc.vector.tensor_copy(oh[:], po[hp][:])
                    nc.gpsimd.dma_start(
                        ot_sb[bi][hp * 64:hp * 64 + 64, q0:q0 + QT],
                        oh[0:DEPTH, :])
                    src = oh[DEPTH:DEPTH + 1, :].rearrange(
                        "o (p j) -> o p j", j=4)
                    nc.gpsimd.dma_start(rs_sb[:, ht * 4:(ht + 1) * 4], src)
                    last_oh[(bi, t, hp)] = oh

            # normalize in two phases: prep (gathers + reciprocal +
            # partition broadcast; cheap queue entries, long latency) right
            # after the attention block, and the [128,512] multiply much
            # later -- so the multiply never head-of-line blocks the next
            # attention block's DVE work while the broadcast chain runs.
            bcbs = {}

            def emit_norm_prep(bi, t):
                bcb = rowp.tile([128, QT], F32, tag="bcb",
                                name=f"bcb{bi}{t}")
                bcbs[(bi, t)] = bcb
                for hp in range(2):
                    h = 2 * bi + hp
                    c0 = (h * NQT + t) * 4
                    nc.vector.reciprocal(rr_sb[:, c0:c0 + 4],
                                         rs_sb[:, c0:c0 + 4])
                    rowh = rowp.tile([1, QT], F32, tag="rowh",
                                     name=f"rowh{bi}{t}{hp}")
                    nc.gpsimd.dma_start(
                        rowh[:].rearrange("o (p j) -> o p j", j=4),
                        rr_sb[:, c0:c0 + 4])
                    if hp == 0:
                        nc.gpsimd.partition_broadcast(bcb[0:64, :], rowh[:])
                    else:
                        tmp = rowp.tile([64, QT], F32, tag="tmp",
                                        name=f"tmp{bi}{t}")
                        nc.gpsimd.partition_broadcast(tmp[:], rowh[:])
                        nc.gpsimd.dma_start(bcb[64:128, :], tmp[:])

            def emit_norm_mul(bi, t, split=False):
                bcb = bcbs.pop((bi, t))
                csl = slice(t * QT, (t + 1) * QT)
                if split:
                    for hp in range(2):
                        rs = slice(hp * 64, (hp + 1) * 64)
                        nc.vector.tensor_mul(ot_sb[bi][rs, csl],
                                             ot_sb[bi][rs, csl],
                                             bcb[rs, :])
                else:
                    nc.vector.tensor_mul(ot_sb[bi][:, csl],
                                         ot_sb[bi][:, csl], bcb[:])

            def emit_norm_tail(bi, t):
                # latency-optimized final normalize: reciprocal directly on
                # the fp16 denominator rows (partition 64 of the extraction
                # tiles), broadcast across partitions via two PE matmuls
                # into a free logits-pool PSUM bank, then one multiply.
                bcb_ps = plg.tile([128, 1024], F32, tag="lg",
                                  name=f"bcbps{bi}{t}")
                csl = slice(t * QT, (t + 1) * QT)
                for hp in range(2):
                    oh = last_oh[(bi, t, hp)]
                    r64 = rowp.tile([DEPTH + 1, QT], mybir.dt.float32r,
                                    tag="r64", name=f"r64{bi}{t}{hp}")
                    with nc.allow_low_precision(
                            reason="fp32r reciprocal feeding the "
                                   "broadcast matmul; fp16 pipeline"):
                        nc.vector.reciprocal(r64[DEPTH:DEPTH + 1, :],
                                             oh[DEPTH:DEPTH + 1, :])
                    nc.tensor.matmul(
                        bcb_ps[:, 0:QT],
                        sel64[DEPTH:DEPTH + 1,
                              hp * 128:(hp + 1) * 128].bitcast(
                                  mybir.dt.float32r),
                        r64[DEPTH:DEPTH + 1, :],
                        start=(hp == 0), stop=(hp == 1))
                nc.vector.tensor_mul(ot_sb[bi][:, csl], ot_sb[bi][:, csl],
                                     bcb_ps[:, 0:QT])

            # ---- driver ----------------------------------------------------
            # DMA issue spread across engine sequencers, need-ordered:
            #   sync:   wq, xq0/1 (+masks, wo, later xq3/xk3/xv3, stores)
            #   scalar: wk, xk0/1 (all before the first exp)
            #   gpsimd: wv, xv0/1 (all before its first extraction DMA)
            #   vector: xq2/xk2/xv2 (after the warm-up bias-adds)
            load_w(nc.sync, wq_sb, wq, bq_sb, bq)
            emit_xdma(nc.sync, "q", 0, strip_ch=(0, 1, 2, 3))
            load_w(nc.scalar, wk_sb, wk, bk_sb, bk)
            emit_xdma(nc.scalar, "k", 0, strip_ch=(0, 1, 2, 3))
            load_w(nc.gpsimd, wv_sb, wv, bv_sb, bv)
            emit_xdma(nc.gpsimd, "v", 0)
            emit_xdma(nc.sync, "q", 1)
            emit_xdma(nc.scalar, "k", 1)
            emit_xdma(nc.gpsimd, "v", 1)
            for i in range(n_masks):
                nc.sync.dma_start(mask_sb[i][:], masks[i])
            for c in range(2):
                nc.sync.dma_start(wo_sb[:, c, :], wo[c * 128:(c + 1) * 128, :])

            nc.vector.memset(ones_f16[:], 1.0)
            nc.vector.memset(sel64[:], 0.0)
            nc.vector.memset(sel64[DEPTH:DEPTH + 1, 0:DEPTH], 1.0)
            nc.vector.memset(sel64[DEPTH:DEPTH + 1, 128 + DEPTH:256], 1.0)
            nc.vector.tensor_copy(
                v_sb[:, :, :, DEPTH:DEPTH + 1],
                ones_f16[:, None, None, :].broadcast_to(
                    [128, NKB, HEADS_PER_CORE, 1]))

            F = _Filler()
            # dedicated warm-up: keeps PE streaming while xv lands
            for th in (proj_quarter_thunks("q", 0)
                       + proj_quarter_thunks("k", 0)
                       + proj_quarter_thunks("q", 1)
                       + proj_quarter_thunks("k", 1)
                       + v_quarter_thunks(0)):
                th()
            emit_xdma(nc.sync, "q", 2)
            emit_xdma(nc.sync, "k", 2)
            emit_xdma(nc.sync, "v", 2)

            # Filler thunks must be fully drained before the first
            # attention pair that READS what they produce (a later-queued
            # producer behind a waiting PE consumer would deadlock); the
            # in-block barriers sit at exactly that pair index.
            F.add(v_quarter_thunks(1))
            F.add(proj_quarter_thunks("q", 2))
            emit_attention(0, 0, F)
            emit_norm_prep(0, 0)
            emit_xdma(nc.sync, "q", 3)
            emit_xdma(nc.sync, "k", 3)
            emit_attention(1, 0, F)
            emit_norm_prep(1, 0)
            emit_xdma(nc.sync, "v", 3)

            # att(0,1) reads v quarter 1 from pair 2 (kb 4+)
            emit_attention(0, 1, F, barrier_at=2)
            emit_norm_prep(0, 1)
            F.add(proj_quarter_thunks("k", 2))
            F.add(v_quarter_thunks(2))
            F.add(proj_quarter_thunks("q", 3))
            emit_attention(1, 1, F)
            emit_norm_prep(1, 1)
            emit_norm_mul(0, 0)
            emit_norm_mul(1, 0)

            F.add(outproj_thunks(0))
            # att(0,2) reads kt/v quarter 2 from pair 4 (kb 8+)
            emit_attention(0, 2, F, barrier_at=4)
            emit_norm_prep(0, 2)
            F.add(proj_quarter_thunks("k", 3))
            F.add(v_quarter_thunks(3))
            emit_attention(1, 2, F)
            emit_norm_prep(1, 2)
            emit_norm_mul(0, 1)
            emit_norm_mul(1, 1)

            F.add(outproj_thunks(1))
            # att(0,3) reads kt/v quarter 3 from pair 6 (kb 12+)
            emit_attention(0, 3, F, barrier_at=6)
            emit_norm_prep(0, 3)
            emit_norm_mul(0, 2)
            emit_norm_mul(1, 2)
            F.add(outproj_thunks(2))
            emit_attention(1, 3, F)
            F.drain_all()
            emit_norm_mul(0, 3)
            emit_norm_tail(1, 3)
            for th in outproj_thunks(3, tail=True):
                th()

    nc.compile()
    return nc


def _plan_from_mask(mask):
    """Classify (qtile, kblock-pair) blocks; returns plan + unique tiles.

    plan[(t, pj)] = (desc0, desc1, mask_idx|None), desc = None (fully
    masked) | (vlo, mask_cols|None): vlo = 128-aligned count of leading
    fully-masked query columns (trimmed everywhere), mask_cols = (cl,ch)
    128-aligned query col range (>= vlo) needing the multiplicative
    mask. Mask tiles are [128, 1024] exp(NEG*mask) of transposed halves.
    """
    m = np.asarray(mask, dtype=np.float64).reshape(S, S)  # [q, k]
    plan = {}
    tiles = []
    keys = {}
    for t in range(NQT):
        first_valid = True
        for pj in range(NPAIR):
            descs = []
            tile_halves = [None, None]
            for half in range(2):
                kb = 2 * pj + half
                blk = m[t * QT:(t + 1) * QT, kb * KB:(kb + 1) * KB]  # [q,k]
                if (blk >= 1.0).all():
                    descs.append(None)
                    continue
                row_full = (blk >= 1.0).all(axis=1)  # fully-masked q rows
                vlo = 0
                while vlo + 128 <= QT and row_full[vlo:vlo + 128].all():
                    vlo += 128
                if first_valid:
                    vlo = 0
                first_valid = False
                rest = blk[vlo:, :]
                if not rest.any():
                    descs.append((vlo, None))
                    continue
                idx = np.nonzero(blk[vlo:, :].any(axis=1))[0] + vlo
                cl = (idx.min() // 128) * 128
                ch = ((idx.max() // 128) + 1) * 128
                descs.append((vlo, (int(cl), int(ch))))
                tile_halves[half] = np.exp(
                    np.float64(NEG) * blk.T).astype(np.float32)
            d0, d1 = descs
            if d0 is None and d1 is None:
                plan[(t, pj)] = (None, None, None)
                continue
            if tile_halves[0] is None and tile_halves[1] is None:
                plan[(t, pj)] = (d0, d1, None)
                continue
            pair = np.zeros((KB, 2 * QT), np.float32)
            for half in range(2):
                hv = tile_halves[half]
                if hv is not None:
                    pair[:, half * QT:(half + 1) * QT] = hv
                elif descs[half] is not None:
                    pair[:, half * QT:(half + 1) * QT] = 1.0
            key = pair.tobytes()
            if key not in keys:
                keys[key] = len(tiles)
                tiles.append(pair)
            plan[(t, pj)] = (d0, d1, keys[key])
    return plan, tiles


def kernel(query, key_in, value, mask, wq, bq, wk, bk, wv, bv, wo, bo):
    query = np.asarray(query, dtype=np.float32)
    key_in = np.asarray(key_in, dtype=np.float32)
    value = np.asarray(value, dtype=np.float32)
    wq = np.asarray(wq, dtype=np.float32)
    wk = np.asarray(wk, dtype=np.float32)
    wv = np.asarray(wv, dtype=np.float32)
    wo = np.asarray(wo, dtype=np.float32)
    bq = np.asarray(bq, dtype=np.float32)
    bk = np.asarray(bk, dtype=np.float32)
    bv = np.asarray(bv, dtype=np.float32)
    bo = np.asarray(bo, dtype=np.float32)

    plan, mask_tiles = _plan_from_mask(mask)
    sig = tuple(sorted(plan.items()))
    if sig not in _cache:
        _cache[sig] = _build(plan, len(mask_tiles))
    nc = _cache[sig]

    scale = 1.0 / np.sqrt(np.float32(DEPTH))
    masks_arr = (np.stack(mask_tiles).astype(np.float16) if mask_tiles
                 else np.zeros((1, KB, 2 * QT), np.float16))

    xT = {}
    for b in range(B):
        xT[("q", b)] = np.ascontiguousarray(query[b].T).astype(np.float16)
        xT[("k", b)] = np.ascontiguousarray(key_in[b].T).astype(np.float16)
        xT[("v", b)] = np.ascontiguousarray(value[b].T).astype(np.float16)

    in_maps = []
    for c in range(N_CORES):
        b = c // CORES_PER_BATCH
        g = c % CORES_PER_BATCH
        sl = slice(g * DC, (g + 1) * DC)
        in_maps.append({
            "xqT": xT[("q", b)],
            "xkT": xT[("k", b)],
            "xvT": xT[("v", b)],
            "wq": (np.ascontiguousarray(wq[:, sl]) * scale).astype(np.float16),
            "wk": np.ascontiguousarray(wk[:, sl]).astype(np.float16),
            "wv": np.ascontiguousarray(wv[:, sl]).astype(np.float16),
            "wo": np.ascontiguousarray(wo[sl, :]).astype(np.float16),
            "bq": np.ascontiguousarray((bq[sl] * scale).reshape(2, 128).T),
            "bk": np.ascontiguousarray(bk[sl].reshape(2, 128).T),
            "bv": np.ascontiguousarray(
                np.broadcast_to(bv[sl], (128, DC))),
            "masks": masks_arr,
        })

    res = run_bass_kernel_spmd(nc, in_maps, list(range(N_CORES)))
    kernel.last_results = res

    out = np.zeros((B, S, D_MODEL), np.float32)
    for b in range(B):
        acc = np.zeros((D_MODEL, S), np.float32)
        for g in range(CORES_PER_BATCH):
            acc += res.results[b * CORES_PER_BATCH + g]["outT"].astype(
                np.float32)
        out[b] = acc.T + bo
    return out
